# revision 23
# baseline (speedup 1.0000x reference)
"""Trainium2 Bass kernel for nn_MultiHeadAttention_47485158424810.

Full-input contract: kernel(**inputs) takes the unsharded numpy inputs and
returns the full [2, 2048, 1024] output.

Sharding (8 cores): core = b*4 + hg
  - data parallel over batch b in {0,1}
  - tensor parallel over 4 head-groups hg (4 heads of 64 dims each -> 256
    output dims per core) by splitting Wq/Wk/Wv rows (column-parallel) and
    Wo columns (row-parallel).  Each core emits a partial [2048, 1024]
    output; the host sums the 4 partials per batch and adds Wo_b.

Device-side plan per core (T=2048, K=1024, O=256, 4 heads of s=64):
  phase 1: stream x in 4 chunks of 512 tokens; PE-transpose to x^T tiles
           [128k, 512t]; project Q^T,K^T ([256, 2048], heads packed at
           partition offsets 0/64) and V (natural [t, o] layout, per-head
           tiles [128, 65] with a ones column for softmax row sums).
           Matmuls in float32r (full PE rate for moving dim >= 256).
  phase 2: per (head, q-chunk of 512): S^T tiles [128 t_k, 512 t_q] on PE,
           exp on ACT (scale=1/32 folded in), causal mask via gpsimd
           affine_select on the 4 diagonal tiles, then attn@V on PE with
           the ones column producing softmax denominators for free.
  phase 3: normalize O^T by broadcasted reciprocal row sums, add V-bias
           (exact: softmax rows sum to 1 => +bv per head dim), then the
           Wo row-parallel matmul and DMA of the partial output.
"""

import os
import sys

import numpy as np

for _p in ("/root/.axon_site/_ro/trn_rl_repo", "/opt/trn_rl_repo"):
    if os.path.isdir(_p) and _p not in sys.path:
        sys.path.append(_p)

import concourse.bass as bass
import concourse.tile as tile
from concourse import bacc, mybir
from concourse.bass_utils import run_bass_kernel_spmd

B, T, K, H = 2, 2048, 1024, 16
NCORES = 8
O = 256  # head-group width per core (4 heads x 64)
S = 64  # head dim
HPC = 4  # heads per core
F32 = mybir.dt.float32
F32R = mybir.dt.float32r
AF = mybir.ActivationFunctionType
ALU = mybir.AluOpType

_CACHE = {}


def _build_body(nc, tc, d, loop_n=0):
    if loop_n:
        with tc.For_i(0, loop_n, 1):
            with tc.tile_pool(name="consts", bufs=1) as consts, \
                 tc.tile_pool(name="persist", bufs=1) as persist, \
                 tc.tile_pool(name="pss", bufs=2, space="PSUM") as pss_p:
                _build_inner(nc, tc, d, consts, persist, pss_p)
        return
    with tc.tile_pool(name="consts", bufs=1) as consts, \
         tc.tile_pool(name="persist", bufs=1) as persist, \
         tc.tile_pool(name="pss", bufs=2, space="PSUM") as pss_p:
        _build_inner(nc, tc, d, consts, persist, pss_p)


def _build_inner(nc, tc, d, consts, persist, pss_p):
    f32 = F32
    x_d, wq_d, wk_d, wv_d, wo_d, bq_d, bk_d, bv_d, y_d = (
        d["x"], d["wqT"], d["wkT"], d["wvT"], d["woT"],
        d["bq"], d["bk"], d["bv"], d["y"],
    )
    def load_wT(ap_d, prefix):
        tiles = []
        for kk in range(8):
            t_ = consts.tile([128, O], F32R, name=f"{prefix}{kk}")
            nc.scalar.dma_start(t_, ap_d[kk * 128:(kk + 1) * 128, :])
            tiles.append(t_)
        return tiles

    wq_sb = load_wT(wq_d, "wq")
    wk_sb = load_wT(wk_d, "wk")
    wv_sb = load_wT(wv_d, "wv")
    wo_sb = []
    for oc in range(2):
        t_ = consts.tile([128, K], F32R, name=f"wo{oc}")
        nc.gpsimd.dma_start(t_, wo_d[oc * 128:(oc + 1) * 128, :])
        wo_sb.append(t_)

    def load_bias(ap_d, nm):
        t_ = consts.tile([128, 2], f32, name=nm)
        nc.gpsimd.dma_start(t_, ap_d.rearrange("(c p) -> p c", p=128))
        return t_

    # causal masks for the 4 diagonal sub-tile offsets m:
    # keep P^T[i, j] where i + 128*m <= j
    masks = []
    for m_ in range(4):
        mt = consts.tile([128, 1024], mybir.dt.bfloat16, name=f"mask{m_}")
        nc.gpsimd.memset(mt, 1.0)
        mt3 = mt.rearrange("p (e j) -> p e j", e=2)
        nc.gpsimd.affine_select(
            out=mt3, in_=mt3, pattern=[[0, 2], [1, 512]],
            compare_op=ALU.is_ge, fill=0.0, base=-128 * m_,
            channel_multiplier=-1)
        masks.append(mt)

    bq_sb = load_bias(bq_d, "bq_sb")
    bk_sb = load_bias(bk_d, "bk_sb")
    bv_sb = load_bias(bv_d, "bv_sb")

    # persistent activations
    qT = [persist.tile([128, T], F32R, name=f"qT{oc}") for oc in range(2)]
    kT = [persist.tile([128, T], F32R, name=f"kT{oc}") for oc in range(2)]
    oT = [persist.tile([128, T], F32R, name=f"oT{oc}") for oc in range(2)]
    # V natural layout, per t_k tile: 4 heads x (64 dims + ones col)
    vv = [persist.tile([128, HPC * (S + 1)], F32R, name=f"v{i}")
          for i in range(T // 128)]
    # softmax denominators, one [1, T] tile per head (base partition 0)
    rsum = [persist.tile([1, T], F32R, name=f"rsum{h}") for h in range(HPC)]
    ones128 = persist.tile([128, 128], F32R, name="ones128")
    ones_f32 = persist.tile([128, 128], f32, name="ones_f32")
    nc.gpsimd.memset(ones_f32, 1.0)
    # f32r tiles can't be memset directly; DVE copy rounds f32 -> f32r
    nc.vector.tensor_copy(ones128, ones_f32)

    for i in range(T // 128):
        # ones column at offset h*(S+1)+S for each head
        nc.vector.tensor_copy(vv[i][:, S::S + 1], ones_f32[:, 0:HPC])

    # ------------- fused streaming loop over 512-token chunks -------------
    # per chunk c: project Q/K/V(c), attend q-chunk c against k-chunks 0..c,
    # normalize, output-project and DMA out.  No phase barriers; every
    # engine pipelines across the c loop.
    inv_scale = 1.0 / float(np.sqrt(K))
    with tc.tile_pool(name="xTp", bufs=2) as xT_p, \
         tc.tile_pool(name="ppr", bufs=2, space="PSUM") as ppr_p, \
         tc.tile_pool(name="pso", bufs=1, space="PSUM") as pso_p, \
         tc.tile_pool(name="ystg", bufs=3) as ystg_p, \
         tc.tile_pool(name="ptile", bufs=3) as pt_p:
        for c in range(4):  # chunks of 512 tokens
            # ---- projections for chunk c ----
            xT = [
                xT_p.tile([128, 512], F32R, name=f"xT{c}_{kk}", tag=f"xT{kk}")
                for kk in range(8)
            ]
            for kk in range(8):
                nc.sync.dma_start(
                    xT[kk],
                    x_d[kk * 128:(kk + 1) * 128, c * 512:(c + 1) * 512])
            # Q^T and K^T: [o on partitions, t free]
            for w_sb, b_sb, dest in ((wq_sb, bq_sb, qT), (wk_sb, bk_sb, kT)):
                for oc in range(2):
                    ps = ppr_p.tile([128, 512], f32, name="ps_qk", tag="ps")
                    for kk in range(8):
                        nc.tensor.matmul(
                            ps,
                            w_sb[kk][:, oc * 128:(oc + 1) * 128],
                            xT[kk],
                            start=(kk == 0), stop=(kk == 7))
                    nc.scalar.activation(
                        dest[oc][:, c * 512:(c + 1) * 512], ps,
                        AF.Identity, bias=b_sb[:, oc:oc + 1])
            # V natural: [t on partitions, o free]; no bias (folded later)
            for a in range(4):
                ps = ppr_p.tile([128, O], f32, name="ps_v", tag="ps")
                for kk in range(8):
                    nc.tensor.matmul(
                        ps,
                        xT[kk][:, a * 128:(a + 1) * 128],
                        wv_sb[kk],
                        start=(kk == 0), stop=(kk == 7))
                for h in range(HPC):
                    nc.vector.tensor_copy(
                        vv[c * 4 + a][:, h * (S + 1):h * (S + 1) + S],
                        ps[:, h * S:(h + 1) * S])

            # ---- attention for q-chunk c ----
            for oc in range(2):  # head pair (2*oc, 2*oc+1)
                po = [pso_p.tile([S + 1, 512], f32, name=f"po{e}",
                                 tag=f"po{e}") for e in range(2)]
                nr = 4 * (c + 1)  # causal: t_k tiles 0..4c+3
                for r in range(nr):
                    m = r - 4 * c
                    j0 = 128 * m if m > 0 else 0  # fully-masked cols skipped
                    # one [128, 1024] psum tile holds S^T for BOTH heads of
                    # the pair; their matmuls use disjoint 64-partition row
                    # groups so the PE overlaps them.
                    ps = pss_p.tile([128, 1024], f32, name="ps_s", tag="pss")
                    for e in range(2):
                        hb = e * 64
                        nc.tensor.matmul(
                            ps[:, e * 512 + j0:(e + 1) * 512],
                            kT[oc][hb:hb + 64, r * 128:(r + 1) * 128],
                            qT[oc][hb:hb + 64, c * 512 + j0:(c + 1) * 512],
                            start=True, stop=True)
                    pt = pt_p.tile([128, 1024], F32R, name="pt_exp", tag="ptl")
                    ps3 = ps.rearrange("p (e j) -> p e j", e=2)[:, :, j0:]
                    pt3 = pt.rearrange("p (e j) -> p e j", e=2)[:, :, j0:]
                    # exp over both heads in one ACT op (ACT is the
                    # bottleneck engine of the attention inner loop)
                    nc.scalar.activation(pt3, ps3, AF.Exp, scale=inv_scale)
                    if m >= 0:
                        mk = masks[m].rearrange("p (e j) -> p e j", e=2)[:, :, j0:]
                        nc.vector.tensor_mul(pt3, pt3, mk)
                    for e in range(2):
                        h = 2 * oc + e
                        nc.tensor.matmul(
                            po[e][:, j0:],
                            vv[r][:, h * (S + 1):(h + 1) * (S + 1)],
                            pt[:, e * 512 + j0:(e + 1) * 512],
                            start=(r == 0), stop=(r == nr - 1))
                # evict: O^T rows (unnormalized) + denominator row.
                # DMA can't read PSUM, so stage via DVE, then shift
                # partitions with SBUF->SBUF DMA (gpsimd queue).
                for e in range(2):
                    h = 2 * oc + e
                    hb = e * 64
                    stg = pt_p.tile([S + 1, 512], F32R, name=f"ostg{e}",
                                    tag=f"ostg{e}", bufs=2)
                    nc.vector.tensor_copy(stg, po[e])
                    nc.gpsimd.dma_start(
                        oT[oc][hb:hb + 64, c * 512:(c + 1) * 512], stg[0:S, :])
                    nc.gpsimd.dma_start(
                        rsum[h][0:1, c * 512:(c + 1) * 512], stg[S:S + 1, :])

                # ---- normalize q-chunk c of this head pair ----
                with nc.allow_low_precision(reason="f32r softmax recip"):
                    for e in range(2):
                        h = 2 * oc + e
                        nc.vector.reciprocal(
                            rsum[h][0:1, c * 512:(c + 1) * 512],
                            rsum[h][0:1, c * 512:(c + 1) * 512])
                for e in range(2):
                    h = 2 * oc + e
                    hb = e * 64
                    # broadcast recip across partitions: ones col x recip row
                    prb = ppr_p.tile([128, 512], f32, name="prb", tag="ps")
                    nc.tensor.matmul(
                        prb, ones128[0:1, :],
                        rsum[h][0:1, c * 512:(c + 1) * 512],
                        start=True, stop=True)
                    nc.vector.tensor_mul(
                        oT[oc][hb:hb + 64, c * 512:(c + 1) * 512],
                        oT[oc][hb:hb + 64, c * 512:(c + 1) * 512],
                        prb[hb:hb + 64, :])
                # + V bias: exact since softmax rows sum to 1
                nc.scalar.activation(
                    oT[oc][:, c * 512:(c + 1) * 512],
                    oT[oc][:, c * 512:(c + 1) * 512],
                    AF.Identity, bias=bv_sb[:, oc:oc + 1])

            # ---- output projection + DMA for t-chunk group c ----
            for i in range(4 * c, 4 * c + 4):
                ys = ystg_p.tile([128, K], f32, name="ystg", tag="ystg")
                for jc in range(2):
                    py = ppr_p.tile([128, 512], f32, name="py", tag="ps")
                    for oc in range(2):
                        nc.tensor.matmul(
                            py,
                            oT[oc][:, i * 128:(i + 1) * 128],
                            wo_sb[oc][:, jc * 512:(jc + 1) * 512],
                            start=(oc == 0), stop=(oc == 1))
                    nc.vector.tensor_copy(ys[:, jc * 512:(jc + 1) * 512], py)
                nc.sync.dma_start(y_d[i * 128:(i + 1) * 128, :], ys)


def build_program(loop_n=0):
    nc = bacc.Bacc("TRN2", target_bir_lowering=False, debug=False,
                   num_devices=NCORES)
    d = {
        "x": nc.dram_tensor("xT", [K, T], F32R, kind="ExternalInput").ap(),
        "wqT": nc.dram_tensor("wqT", [K, O], F32R, kind="ExternalInput").ap(),
        "wkT": nc.dram_tensor("wkT", [K, O], F32R, kind="ExternalInput").ap(),
        "wvT": nc.dram_tensor("wvT", [K, O], F32R, kind="ExternalInput").ap(),
        "woT": nc.dram_tensor("woT", [O, K], F32R, kind="ExternalInput").ap(),
        "bq": nc.dram_tensor("bq", [O], F32, kind="ExternalInput").ap(),
        "bk": nc.dram_tensor("bk", [O], F32, kind="ExternalInput").ap(),
        "bv": nc.dram_tensor("bv", [O], F32, kind="ExternalInput").ap(),
        "y": nc.dram_tensor("y", [T, K], F32, kind="ExternalOutput").ap(),
    }
    with tile.TileContext(nc) as tc:
        _build_body(nc, tc, d, loop_n=loop_n)
    nc.compile()
    return nc


def _get_program():
    if "nc" not in _CACHE:
        _CACHE["nc"] = build_program()
    return _CACHE["nc"]


def make_in_maps(x, Wq_w, Wk_w, Wv_w, Wo_w, Wq_b, Wk_b, Wv_b):
    in_maps = []
    for core in range(NCORES):
        b, hg = divmod(core, 4)
        sl = slice(hg * O, (hg + 1) * O)
        in_maps.append({
            "xT": np.ascontiguousarray(x[b].T, np.float32),
            "wqT": np.ascontiguousarray(Wq_w[sl, :].T, np.float32),
            "wkT": np.ascontiguousarray(Wk_w[sl, :].T, np.float32),
            "wvT": np.ascontiguousarray(Wv_w[sl, :].T, np.float32),
            "woT": np.ascontiguousarray(Wo_w[:, sl].T, np.float32),
            "bq": np.ascontiguousarray(Wq_b[sl], np.float32),
            "bk": np.ascontiguousarray(Wk_b[sl], np.float32),
            "bv": np.ascontiguousarray(Wv_b[sl], np.float32),
        })
    return in_maps


def _combine(results, Wo_b):
    y = np.empty((B, T, K), np.float32)
    for b in range(B):
        acc = results[b * 4]["y"].copy()
        for hg in range(1, 4):
            acc += results[b * 4 + hg]["y"]
        y[b] = acc + np.asarray(Wo_b, np.float32)
    return y


def kernel(x, Wq_w, Wq_b, Wk_w, Wk_b, Wv_w, Wv_b, Wo_w, Wo_b, _trace=False):
    x = np.asarray(x, np.float32)
    nc = _get_program()
    in_maps = make_in_maps(x, np.asarray(Wq_w), np.asarray(Wk_w),
                           np.asarray(Wv_w), np.asarray(Wo_w),
                           np.asarray(Wq_b), np.asarray(Wk_b),
                           np.asarray(Wv_b))
    out = run_bass_kernel_spmd(nc, in_maps, list(range(NCORES)),
                               trace=_trace)
    _CACHE["last_exec_ns"] = out.exec_time_ns
    return _combine(out.results, Wo_b)
